# revision 2
# baseline (speedup 1.0000x reference)
"""ChiSquareLoss kernel for Trainium2 (8 NeuronCores, SPMD).

Problem (see reference): for each of B=16384 rows of a [B, 2048] f32 matrix,
build a 10-bin histogram between the row's min and max, then
chi2_row = sum_j (obs_j - e)^2 / (e + eps) with e = B/10, and return
mean(chi2_row).

Key perf fact (cost model + HW): DVE tensor_scalar WITH accum_out still
supports the 2x/4x perf modes (the accumulator AP is a free_size-1 operand
and exempt from the 2-byte dtype rule), while scalar_tensor_tensor supports
none.  So a plain bf16 tensor_scalar(is_gt)+accum pass counts one boundary
in ~594ns ((58+2048/4)/0.96), vs ~2.2us for any stt/1x formulation.

Plan per row (each core handles B/8 = 2048 rows, 16 tiles of [128, 2048]):
  c_k = #{x in row : x > b_k},  b_k = mn + (mx-mn)*k/10, k=1..9
  obs_j = c_j - c_{j+1}, c_0 = 2048, c_10 = 0.
Input is host-cast to bf16 (halves DMA; classification of bf16(x) against
f32 boundaries perturbs only elements within half a bf16 ulp of a boundary
-- measured end-to-end rel err ~1e-6).

Engine split per [128, 2048] bf16 tile:
  DVE : row max, row min (tensor_scalar mult/max|min accum, 4x, 594ns)
        7 counting passes (tensor_scalar is_gt/add accum, 4x, 594ns)
  ACT : delta = mx-mn, b_k vector (Identity with AP scale/bias, ~300ns each)
        2 counting passes via Sign(b_k - x) + sum accum (~2.2us each)
One-tile software pipeline: tile t's counts are emitted after tile t+1's
minmax/boundary ops so the cross-engine chain (DVE minmax -> ACT b_k ->
counts) never stalls either engine.
Epilogue: ACT sign-sums -> counts, difference into obs, one ACT
Square(obs - e) pass with accumulator -> per-partition partial sums.
Host: total / (e + eps) / B.
"""

import numpy as np

_B_FULL = 16384
_D = 2048
_N_CORES = 8
_ROWS_PER_CORE = _B_FULL // _N_CORES  # 2048
_P = 128
_TILES = _ROWS_PER_CORE // _P  # 16
_BINS = 10
# reference: expected = f32(B/BINS); expected + 1e-8 rounds back to the same f32
_E_F32 = np.float32(_B_FULL / _BINS)  # 1638.4f

_N_ACT = 2                    # boundaries counted on ACT (via Sign)
_N_DVE = 9 - _N_ACT           # boundaries counted on DVE (via is_gt)
_DVE_KS = list(range(1, _N_DVE + 1))          # k = 1..7
_ACT_KS = list(range(_N_DVE + 1, 10))         # k = 8, 9

_CACHE = {}


def _build_program():
    import concourse.bacc as bacc
    import concourse.mybir as mybir
    import concourse.tile as tile

    f32 = mybir.dt.float32
    bf16 = mybir.dt.bfloat16
    Alu = mybir.AluOpType
    Act = mybir.ActivationFunctionType

    nc = bacc.Bacc(None, target_bir_lowering=False)
    x = nc.dram_tensor("x", [_ROWS_PER_CORE, _D], bf16, kind="ExternalInput")
    out = nc.dram_tensor("partial", [_P, 1], f32, kind="ExternalOutput")

    T = _TILES
    # fracs exactly as the reference: f32(k)/f32(10)
    fr = [float(np.float32(k) / np.float32(10.0)) for k in range(1, 10)]

    with tile.TileContext(nc) as tc:
        with tc.tile_pool(name="singles", bufs=1) as singles, \
             tc.tile_pool(name="xp", bufs=4) as xpool, \
             tc.tile_pool(name="small", bufs=3) as small:

            # persistent accumulators
            dacc = singles.tile([_P, T * _N_DVE], f32)   # DVE counts
            sacc = singles.tile([_P, T * _N_ACT], f32)   # ACT sign sums
            c_all = singles.tile([_P, T * 11], f32)      # c_0..c_10 per tile
            fracs = singles.tile([_P, 9], f32)           # k/10
            ebias = singles.tile([_P, 1], f32)           # -e
            dscr = singles.tile([_P, _D], bf16)          # DVE scratch out
            ascr = singles.tile([_P, _D], bf16)          # ACT scratch out
            c3 = c_all[:].rearrange("p (t k) -> p t k", k=11)
            nc.gpsimd.memset(c3[:, :, 0:1], float(_D))   # c_0 = 2048
            nc.gpsimd.memset(c3[:, :, 10:11], 0.0)       # c_10 = 0
            for i, f in enumerate(fr):
                nc.gpsimd.memset(fracs[:, i:i + 1], f)
            nc.gpsimd.memset(ebias[:], -float(_E_F32))

            def counts_for(t, st):
                xt, bpos = st
                for i, k in enumerate(_DVE_KS):
                    col = t * _N_DVE + i
                    nc.vector.tensor_scalar(
                        dscr[:], xt[:], bpos[:, k - 1:k], None,
                        Alu.is_gt, Alu.add, accum_out=dacc[:, col:col + 1])
                for i, k in enumerate(_ACT_KS):
                    slot = t * _N_ACT + i
                    nc.scalar.activation(
                        ascr[:], xt[:], Act.Sign, bias=bpos[:, k - 1:k],
                        scale=-1.0, accum_out=sacc[:, slot:slot + 1])

            # one-tile software pipeline (see module docstring)
            prev = None
            for t in range(T):
                xt = xpool.tile([_P, _D], bf16, tag="xt")
                nc.sync.dma_start(out=xt[:], in_=x[t * _P:(t + 1) * _P, :])

                mx = small.tile([_P, 1], f32, tag="mx")
                mn = small.tile([_P, 1], f32, tag="mn")
                delta = small.tile([_P, 1], f32, tag="delta")
                bpos = small.tile([_P, 9], f32, tag="bpos")  # b_k

                nc.vector.tensor_scalar(dscr[:], xt[:], 1.0, None,
                                        Alu.mult, Alu.max, accum_out=mx[:])
                nc.vector.tensor_scalar(dscr[:], xt[:], 1.0, None,
                                        Alu.mult, Alu.min, accum_out=mn[:])
                # boundary math on ACT: delta = -mn + mx ; b_k = frac_k*delta + mn
                nc.scalar.activation(delta[:], mn[:], Act.Identity,
                                     bias=mx[:], scale=-1.0)
                nc.scalar.activation(bpos[:], fracs[:], Act.Identity,
                                     bias=mn[:], scale=delta[:])

                if prev is not None:
                    counts_for(t - 1, prev)
                prev = (xt, bpos)
            counts_for(T - 1, prev)

            # ---- epilogue ----
            # ACT sign-sums (sign(b_k - x)) -> counts: c = 1024 - 0.5*S
            conv = singles.tile([_P, T * _N_ACT], f32)
            nc.vector.tensor_scalar(conv[:], sacc[:], -0.5, float(_D // 2),
                                    Alu.mult, Alu.add)
            d3 = dacc[:].rearrange("p (t k) -> p t k", k=_N_DVE)
            v3 = conv[:].rearrange("p (t k) -> p t k", k=_N_ACT)
            nc.vector.tensor_copy(c3[:, :, 1:1 + _N_DVE], d3[:, :, :])
            nc.vector.tensor_copy(c3[:, :, 1 + _N_DVE:10], v3[:, :, :])
            # obs_j = c_j - c_{j+1}
            obs = singles.tile([_P, T * 10], f32)
            obs3 = obs[:].rearrange("p (t j) -> p t j", j=10)
            nc.vector.tensor_tensor(out=obs3[:, :, 0:10], in0=c3[:, :, 0:10],
                                    in1=c3[:, :, 1:11], op=Alu.subtract)

            sq = singles.tile([_P, T * 10], f32)
            part = singles.tile([_P, 1], f32)
            nc.scalar.activation(sq[:], obs[:], Act.Square,
                                 bias=ebias[:], scale=1.0,
                                 accum_out=part[:])
            nc.sync.dma_start(out=out[:], in_=part[:])

    nc.compile()
    return nc


def _get_program():
    if "nc" not in _CACHE:
        _CACHE["nc"] = _build_program()
    return _CACHE["nc"]


def kernel(embeddings: np.ndarray) -> np.ndarray:
    import ml_dtypes
    from concourse.bass_utils import run_bass_kernel_spmd

    assert embeddings.shape == (_B_FULL, _D), embeddings.shape
    x = np.ascontiguousarray(
        embeddings.astype(np.float32).astype(ml_dtypes.bfloat16))
    nc = _get_program()
    in_maps = [
        {"x": x[c * _ROWS_PER_CORE:(c + 1) * _ROWS_PER_CORE]}
        for c in range(_N_CORES)
    ]
    res = run_bass_kernel_spmd(nc, in_maps, core_ids=list(range(_N_CORES)))
    total = np.float64(0.0)
    for r in res.results:
        total += r["partial"].astype(np.float64).sum()
    mean_chi2 = total / np.float64(_E_F32) / np.float64(_B_FULL)
    return np.float32(mean_chi2)


# revision 3
# speedup vs baseline: 1.9861x; 1.9861x over previous
"""ChiSquareLoss kernel for Trainium2 (8 NeuronCores, SPMD).

Problem (see reference): for each of B=16384 rows of a [B, 2048] f32 matrix,
build a 10-bin histogram between the row's min and max, then
chi2_row = sum_j (obs_j - e)^2 / (e + eps) with e = B/10, and return
mean(chi2_row).

Counts per row: c_k = #{x : x > b_k}, b_k = mn + (mx-mn)*k/10 (k=1..9);
obs_j = c_j - c_{j+1}, c_0 = 2048, c_10 = 0.

HW facts (measured): any DVE/ACT instruction with an accumulated [P,1]
output runs at 1x (1 elem/cycle/lane) regardless of dtype -- the
accumulator absorbs one value per lane-cycle.  Masks/tensor_tensor run
at 4x/2x in bf16.  So the cheapest counting primitives are:
  * custom DVE op PAIRCNT_CHI2 (registered at import):
      out = (x > b_lo) + 4096*(x > b_hi); accum = sum(out)
    -> TWO counts per 1x pass (~2.49us incl accumulator read), exact in
    fp32 (counts <= 2047 < 4096).  No separate mask pass needed.
  * ACT Sign(b_k - x) + sum accumulator -> one count per ~2.28us.
  * row min/max: two bf16 tensor_tensor fold levels (2x) + short 1x
    accumulate (~1.88us each vs 2.46us for the direct accumulate).

Engine split per [128, 2048] bf16 tile (16 tiles/core):
  DVE: fold-tree max, fold-tree min, then 2 (even tiles) / 3 (odd tiles)
       PAIRCNT passes.
  ACT: delta = mx-mn, b_k vector, then 5 (even) / 3 (odd) Sign counts.
This alternation balances DVE ~ ACT ~ 9.9us/tile.  One-tile software
pipeline: tile t's counts are emitted after tile t+1's minmax/boundary
ops so the cross-engine chain never stalls either engine.

Input is host-cast to bf16 (classification of bf16(x) against f32
boundaries; measured end-to-end rel err ~1e-5, gate is 2e-2).
Epilogue: unpack pairs (2^23 magic-float floor), convert sign-sums,
difference into obs, one ACT Square(obs - e) accumulated pass ->
per-partition partials; host: total / (e + eps) / B.
"""

import numpy as np

_B_FULL = 16384
_D = 2048
_N_CORES = 8
_ROWS_PER_CORE = _B_FULL // _N_CORES  # 2048
_P = 128
_TILES = _ROWS_PER_CORE // _P  # 16
_BINS = 10
_E_F32 = np.float32(_B_FULL / _BINS)  # 1638.4f
_MAGIC = float(np.float32(2 ** 23 + 2 ** 22))  # round-to-int magic for fp32

_CACHE = {}

# per-tile split: (dve_pairs, act_singles). 2a+c = 9.
_SPLIT_EVEN = ([(1, 2), (3, 4)], [5, 6, 7, 8, 9])
_SPLIT_ODD = ([(1, 2), (3, 4), (5, 6)], [7, 8, 9])


def _register_paircnt():
    """Register the custom DVE op PAIRCNT_CHI2 in concourse.dve_ops:
    out = (x > b_lo) + 4096*(x > b_hi); accum_out = sum(out).
    Uses the documented extension point (append to OPS); sha computed
    at registration so it can never drift."""
    import concourse.dve_ops as dve_ops
    from concourse.dve_ops import DveOp, OPS, _SUB_OPCODE_FOR_NAME, \
        _CUSTOM_DVE_ROW_BASE
    from concourse.dve_spec import Spec, Src0, C0, C1, C2, AluOp, lower
    from concourse.dve_uop import DveOpSpec

    if "PAIRCNT_CHI2" in _SUB_OPCODE_FOR_NAME:
        return next(op for op in OPS if op.name == "PAIRCNT_CHI2")

    def _ref(in0, in1, s0, s1, imm2):
        b = ((in0.astype(np.float32) > s0) +
             (in0.astype(np.float32) > s1) * imm2).astype(np.float32)
        return b, b.reshape(b.shape[0], -1).sum(axis=-1, keepdims=True)

    spec = Spec(
        body=(Src0 > C0) + (Src0 > C1) * C2,
        accum=AluOp.ADD,
        reference=_ref,
    )
    row = _CUSTOM_DVE_ROW_BASE + len(OPS)
    shas = {
        ver: DveOpSpec(name="PAIRCNT_CHI2", opcode=row,
                       uops=lower(spec, ver=ver), rd1_en=False).sha(ver)
        for ver in ("v3", "v4")
    }
    op = DveOp("PAIRCNT_CHI2", spec, subdim=False, uops_sha=shas)
    _SUB_OPCODE_FOR_NAME["PAIRCNT_CHI2"] = row
    OPS.append(op)
    dve_ops.CUSTOM_DVE_SPECS["PAIRCNT_CHI2"] = spec
    return op


def _build_program():
    import concourse.bacc as bacc
    import concourse.mybir as mybir
    import concourse.tile as tile

    f32 = mybir.dt.float32
    bf16 = mybir.dt.bfloat16
    Alu = mybir.AluOpType
    Act = mybir.ActivationFunctionType
    paircnt = _register_paircnt()

    nc = bacc.Bacc(None, target_bir_lowering=False)
    x = nc.dram_tensor("x", [_ROWS_PER_CORE, _D], bf16, kind="ExternalInput")
    out = nc.dram_tensor("partial", [_P, 1], f32, kind="ExternalOutput")

    T = _TILES
    fr = [float(np.float32(k) / np.float32(10.0)) for k in range(1, 10)]

    with tile.TileContext(nc) as tc:
        with tc.tile_pool(name="singles", bufs=1) as singles, \
             tc.tile_pool(name="xp", bufs=4) as xpool, \
             tc.tile_pool(name="small", bufs=3) as small:

            # persistent accumulators: 3 pair slots + 5 sign slots per tile
            pairacc = singles.tile([_P, T * 3], f32)
            sgnacc = singles.tile([_P, T * 5], f32)
            c_all = singles.tile([_P, T * 11], f32)     # c_0..c_10 per tile
            fracs = singles.tile([_P, 9], f32)
            ebias = singles.tile([_P, 1], f32)          # -e
            pscr = singles.tile([_P, _D], f32)          # pair-op out scratch
            ascr = singles.tile([_P, _D], bf16)         # ACT sign out scratch
            f1 = singles.tile([_P, _D // 2], bf16)      # fold scratch L1
            f2 = singles.tile([_P, _D // 4], bf16)      # fold scratch L2
            c3 = c_all[:].rearrange("p (t k) -> p t k", k=11)
            nc.gpsimd.memset(c3[:, :, 0:1], float(_D))  # c_0 = 2048
            nc.gpsimd.memset(c3[:, :, 10:11], 0.0)      # c_10 = 0
            for i, f in enumerate(fr):
                nc.gpsimd.memset(fracs[:, i:i + 1], f)
            nc.gpsimd.memset(ebias[:], -float(_E_F32))
            pa3 = pairacc[:].rearrange("p (t k) -> p t k", k=3)
            sg3 = sgnacc[:].rearrange("p (t k) -> p t k", k=5)
            # zero the unused slots (even tiles: pair slot 2; odd: sign 3,4)
            nc.gpsimd.memset(pa3[:, 0:T:2, 2:3], 0.0)
            nc.gpsimd.memset(sg3[:, 1:T:2, 3:5], 0.0)

            def counts_for(t, st):
                xt, bpos = st
                pairs_t, act_ks = _SPLIT_EVEN if t % 2 == 0 else _SPLIT_ODD
                for pi, (lo, hi) in enumerate(pairs_t):
                    col = t * 3 + pi
                    nc.vector._custom_dve(
                        paircnt, out=pscr[:], in0=xt[:],
                        s0=bpos[:, lo - 1:lo], s1=bpos[:, hi - 1:hi],
                        imm2=4096.0, accum_out=pairacc[:, col:col + 1])
                for i, k in enumerate(act_ks):
                    slot = t * 5 + i
                    nc.scalar.activation(
                        ascr[:], xt[:], Act.Sign, bias=bpos[:, k - 1:k],
                        scale=-1.0, accum_out=sgnacc[:, slot:slot + 1])

            # one-tile software pipeline (see module docstring)
            prev = None
            for t in range(T):
                xt = xpool.tile([_P, _D], bf16, tag="xt")
                nc.sync.dma_start(out=xt[:], in_=x[t * _P:(t + 1) * _P, :])

                mx = small.tile([_P, 1], f32, tag="mx")
                mn = small.tile([_P, 1], f32, tag="mn")
                delta = small.tile([_P, 1], f32, tag="delta")
                bpos = small.tile([_P, 9], f32, tag="bpos")

                # row max / min: two bf16 fold levels (2x) + short accumulate
                nc.vector.tensor_tensor(out=f1[:], in0=xt[:, 0:1024],
                                        in1=xt[:, 1024:2048], op=Alu.max)
                nc.vector.tensor_tensor(out=f2[:], in0=f1[:, 0:512],
                                        in1=f1[:, 512:1024], op=Alu.max)
                nc.vector.tensor_scalar(f2[:], f2[:], 1.0, None,
                                        Alu.mult, Alu.max, accum_out=mx[:])
                nc.vector.tensor_tensor(out=f1[:], in0=xt[:, 0:1024],
                                        in1=xt[:, 1024:2048], op=Alu.min)
                nc.vector.tensor_tensor(out=f2[:], in0=f1[:, 0:512],
                                        in1=f1[:, 512:1024], op=Alu.min)
                nc.vector.tensor_scalar(f2[:], f2[:], 1.0, None,
                                        Alu.mult, Alu.min, accum_out=mn[:])
                # boundary math on ACT: delta = -mn + mx ; b_k = frac_k*delta + mn
                nc.scalar.activation(delta[:], mn[:], Act.Identity,
                                     bias=mx[:], scale=-1.0)
                nc.scalar.activation(bpos[:], fracs[:], Act.Identity,
                                     bias=mn[:], scale=delta[:])

                if prev is not None:
                    counts_for(t - 1, prev)
                prev = (xt, bpos)
            counts_for(T - 1, prev)

            # ---- epilogue ----
            # unpack pairs: hi = round((acc - lo)/4096) via magic float; the
            # magic add of 2^23+2^22 floors acc*2^-12 to an integer exactly
            chi = singles.tile([_P, T * 3], f32)
            clo = singles.tile([_P, T * 3], f32)
            nc.vector.tensor_scalar(chi[:], pairacc[:], float(2.0 ** -12),
                                    _MAGIC, Alu.mult, Alu.add)
            nc.vector.tensor_scalar(chi[:], chi[:], -_MAGIC, None, Alu.add)
            nc.vector.scalar_tensor_tensor(
                out=clo[:], in0=chi[:], scalar=-4096.0, in1=pairacc[:],
                op0=Alu.mult, op1=Alu.add)
            chi3 = chi[:].rearrange("p (t k) -> p t k", k=3)
            clo3 = clo[:].rearrange("p (t k) -> p t k", k=3)
            # pairs (1,2),(3,4) on every tile; (5,6) on odd tiles only
            for pi, (lo, hi) in enumerate([(1, 2), (3, 4)]):
                nc.vector.tensor_copy(c3[:, :, lo:lo + 1], clo3[:, :, pi:pi + 1])
                nc.vector.tensor_copy(c3[:, :, hi:hi + 1], chi3[:, :, pi:pi + 1])
            nc.vector.tensor_copy(c3[:, 1:T:2, 5:6], clo3[:, 1:T:2, 2:3])
            nc.vector.tensor_copy(c3[:, 1:T:2, 6:7], chi3[:, 1:T:2, 2:3])
            # ACT sign-sums (sign(b_k - x)) -> counts: c = 1024 - 0.5*S
            conv = singles.tile([_P, T * 5], f32)
            nc.vector.tensor_scalar(conv[:], sgnacc[:], -0.5, float(_D // 2),
                                    Alu.mult, Alu.add)
            conv3 = conv[:].rearrange("p (t k) -> p t k", k=5)
            nc.vector.tensor_copy(c3[:, 0:T:2, 5:10], conv3[:, 0:T:2, 0:5])
            nc.vector.tensor_copy(c3[:, 1:T:2, 7:10], conv3[:, 1:T:2, 0:3])
            # obs_j = c_j - c_{j+1}
            obs = singles.tile([_P, T * 10], f32)
            obs3 = obs[:].rearrange("p (t j) -> p t j", j=10)
            nc.vector.tensor_tensor(out=obs3[:, :, 0:10], in0=c3[:, :, 0:10],
                                    in1=c3[:, :, 1:11], op=Alu.subtract)

            sq = singles.tile([_P, T * 10], f32)
            part = singles.tile([_P, 1], f32)
            nc.scalar.activation(sq[:], obs[:], Act.Square,
                                 bias=ebias[:], scale=1.0,
                                 accum_out=part[:])
            nc.sync.dma_start(out=out[:], in_=part[:])

    nc.compile()
    return nc


def _get_program():
    if "nc" not in _CACHE:
        _CACHE["nc"] = _build_program()
    return _CACHE["nc"]


def kernel(embeddings: np.ndarray) -> np.ndarray:
    import ml_dtypes
    from concourse.bass_utils import run_bass_kernel_spmd

    assert embeddings.shape == (_B_FULL, _D), embeddings.shape
    x = np.ascontiguousarray(
        embeddings.astype(np.float32).astype(ml_dtypes.bfloat16))
    nc = _get_program()
    in_maps = [
        {"x": x[c * _ROWS_PER_CORE:(c + 1) * _ROWS_PER_CORE]}
        for c in range(_N_CORES)
    ]
    res = run_bass_kernel_spmd(nc, in_maps, core_ids=list(range(_N_CORES)))
    total = np.float64(0.0)
    for r in res.results:
        total += r["partial"].astype(np.float64).sum()
    mean_chi2 = total / np.float64(_E_F32) / np.float64(_B_FULL)
    return np.float32(mean_chi2)


# revision 4
# speedup vs baseline: 2.5468x; 1.2823x over previous
"""ChiSquareLoss kernel for Trainium2 (8 NeuronCores, SPMD).

Problem (see reference): for each of B=16384 rows of a [B, 2048] f32 matrix,
build a 10-bin histogram between the row's min and max, then
chi2_row = sum_j (obs_j - e)^2 / (e + eps) with e = B/10, and return
mean(chi2_row).

Counts per row: c_k = #{x : x > b_k}, b_k = mn + (mx-mn)*k/10 (k=1..9);
obs_j = c_j - c_{j+1}, c_0 = 2048, c_10 = 0.

HW law (measured): any DVE/ACT instruction with an accumulated [P,1]
output runs at 1 output-position per cycle per lane, regardless of
dtype.  The lever is therefore to SHORTEN the accumulated stream and to
PACK counts, via custom DVE microcode ops (concourse.dve_ops extension
point, registered at build time):

  SCANTRIPLE_A : out = scan(+, (S0>C0) + (S1>C0) + (S0>C1)*4096)
  SCANTRIPLE_B : out = scan(+, (S0>C0) + (S1>C0) + (S1>C1)*4096)
  SCANMAX2     : out = scan(max, max(S0, S1))
  SCANMIN2     : out = scan(min, min(S0, S1), init=+FLT_MAX via s0)

Feeding S0/S1 the two halves of a row makes the streamed length 1024
instead of 2048, and the scan's last element IS the fold result -- read
as an AP slice, no accumulator stage or read needed.  One A/B op pair
counts three boundaries (two full + one split across the halves) in
2 x ~1.22us = ~0.82us/count; min/max cost ~1.22us each (vs ~2.4us for
the stock accumulate).  The scan sums stay < 2^23 so fp32 is exact.

Engine split per [128, 2048] bf16 tile (16 tiles/core):
  DVE: SCANMAX2, SCANMIN2, 4 triple ops (boundaries 1..6), and one
       strided [P,4] gather of the scan tails into the pair accumulator.
  ACT: delta = mx-mn, b_k vector, 3 Sign(b_k - x) counts (k=7,8,9).
Both engines ~7.5-7.9us/tile.  One-tile software pipeline: tile t's
counts are emitted after tile t+1's minmax/boundary ops so the
cross-engine chain never stalls either engine.

Input is host-cast to bf16 (classification of bf16(x) against f32
boundaries; measured end-to-end rel err ~1e-5, gate is 2e-2).
Epilogue: unpack triples (2^23 magic-float floor), convert sign-sums,
difference into obs, one ACT Square(obs - e) accumulated pass ->
per-partition partials; host: total / (e + eps) / B.
"""

import numpy as np

_B_FULL = 16384
_D = 2048
_H = _D // 2  # 1024, stream length
_N_CORES = 8
_ROWS_PER_CORE = _B_FULL // _N_CORES  # 2048
_P = 128
_TILES = _ROWS_PER_CORE // _P  # 16
_BINS = 10
_E_F32 = np.float32(_B_FULL / _BINS)  # 1638.4f
_MAGIC = float(np.float32(2 ** 23 + 2 ** 22))  # round-to-int magic for fp32
_FLT_MAX = float(np.finfo(np.float32).max)

_CACHE = {}


def _register_ops():
    """Register the custom DVE scan ops in concourse.dve_ops (documented
    extension point: append to OPS).  sha computed at registration so it
    can never drift.  Idempotent."""
    import concourse.dve_ops as dve_ops
    from concourse.dve_ops import DveOp, OPS, _SUB_OPCODE_FOR_NAME, \
        _CUSTOM_DVE_ROW_BASE
    from concourse.dve_spec import Spec, Src0, Src1, C0, C1, C2, AluOp, \
        maxx, minn, scan, lower
    from concourse.dve_uop import DveOpSpec

    def _reg(name, spec):
        if name in _SUB_OPCODE_FOR_NAME:
            return next(op for op in OPS if op.name == name)
        row = _CUSTOM_DVE_ROW_BASE + len(OPS)
        shas = {
            ver: DveOpSpec(name=name, opcode=row, uops=lower(spec, ver=ver),
                           rd1_en=True).sha(ver)
            for ver in ("v3", "v4")
        }
        op = DveOp(name, spec, subdim=False, uops_sha=shas)
        _SUB_OPCODE_FOR_NAME[name] = row
        OPS.append(op)
        dve_ops.CUSTOM_DVE_SPECS[name] = spec
        return op

    def _r_tri_a(in0, in1, s0, s1, imm2):
        a, b = in0.astype(np.float32), in1.astype(np.float32)
        e = (a > s0) + (b > s0) + (a > s1) * imm2
        return np.cumsum(e.astype(np.float32), -1).astype(np.float32)

    def _r_tri_b(in0, in1, s0, s1, imm2):
        a, b = in0.astype(np.float32), in1.astype(np.float32)
        e = (a > s0) + (b > s0) + (b > s1) * imm2
        return np.cumsum(e.astype(np.float32), -1).astype(np.float32)

    def _r_max(in0, in1, s0, s1, imm2):
        e = np.maximum(in0.astype(np.float32), in1.astype(np.float32))
        return np.maximum.accumulate(e, -1).astype(np.float32)

    def _r_min(in0, in1, s0, s1, imm2):
        e = np.minimum(np.minimum(in0.astype(np.float32),
                                  in1.astype(np.float32)), s0)
        return np.minimum.accumulate(e, -1).astype(np.float32)

    tri_a = _reg("SCANTRIPLE_A_CHI2", Spec(
        body=scan(AluOp.ADD, (Src0 > C0) + (Src1 > C0) + (Src0 > C1) * C2),
        reference=_r_tri_a))
    tri_b = _reg("SCANTRIPLE_B_CHI2", Spec(
        body=scan(AluOp.ADD, (Src0 > C0) + (Src1 > C0) + (Src1 > C1) * C2),
        reference=_r_tri_b))
    smax = _reg("SCANMAX2_CHI2", Spec(
        body=scan(AluOp.MAX, maxx(Src0, Src1)), reference=_r_max))
    smin = _reg("SCANMIN2_CHI2", Spec(
        body=scan(AluOp.MIN, minn(Src0, Src1), init=C0), reference=_r_min))
    return tri_a, tri_b, smax, smin


def _build_program():
    import concourse.bacc as bacc
    import concourse.mybir as mybir
    import concourse.tile as tile

    f32 = mybir.dt.float32
    bf16 = mybir.dt.bfloat16
    Alu = mybir.AluOpType
    Act = mybir.ActivationFunctionType
    tri_a, tri_b, smax, smin = _register_ops()

    nc = bacc.Bacc(None, target_bir_lowering=False)
    x = nc.dram_tensor("x", [_ROWS_PER_CORE, _D], bf16, kind="ExternalInput")
    out = nc.dram_tensor("partial", [_P, 1], f32, kind="ExternalOutput")

    T = _TILES
    fr = [float(np.float32(k) / np.float32(10.0)) for k in range(1, 10)]

    with tile.TileContext(nc) as tc:
        with tc.tile_pool(name="singles", bufs=1) as singles, \
             tc.tile_pool(name="xp", bufs=4) as xpool, \
             tc.tile_pool(name="mm", bufs=3) as mmpool, \
             tc.tile_pool(name="small", bufs=3) as small:

            pairacc = singles.tile([_P, T * 4], f32)   # 4 scan tails / tile
            sgnacc = singles.tile([_P, T * 3], f32)    # 3 sign slots / tile
            c_all = singles.tile([_P, T * 11], f32)    # c_0..c_10 per tile
            fracs = singles.tile([_P, 9], f32)
            ebias = singles.tile([_P, 1], f32)         # -e
            fltmax = singles.tile([_P, 1], f32)        # +FLT_MAX (min-scan seed)
            scnt = singles.tile([_P, 4 * _H], f32)     # triple-op out scratch
            ascr = singles.tile([_P, _D], bf16)        # ACT sign out scratch
            c3 = c_all[:].rearrange("p (t k) -> p t k", k=11)
            nc.gpsimd.memset(c3[:, :, 0:1], float(_D))  # c_0 = 2048
            nc.gpsimd.memset(c3[:, :, 10:11], 0.0)      # c_10 = 0
            for i, f in enumerate(fr):
                nc.gpsimd.memset(fracs[:, i:i + 1], f)
            nc.gpsimd.memset(ebias[:], -float(_E_F32))
            nc.gpsimd.memset(fltmax[:], _FLT_MAX)
            stail = scnt[:].rearrange("p (s n) -> p s n", n=_H)[:, :, _H - 1]

            def counts_for(t, st):
                xt, bpos = st
                x0, x1 = xt[:, 0:_H], xt[:, _H:_D]
                # boundaries 1..6 via two A/B triple pairs: (1,2,3), (4,5,6)
                for g, (ka, kb, kc) in enumerate([(1, 2, 3), (4, 5, 6)]):
                    nc.vector._custom_dve(
                        tri_a, out=scnt[:, (2 * g) * _H:(2 * g + 1) * _H],
                        in0=x0, in1=x1, s0=bpos[:, ka - 1:ka],
                        s1=bpos[:, kb - 1:kb], imm2=4096.0)
                    nc.vector._custom_dve(
                        tri_b, out=scnt[:, (2 * g + 1) * _H:(2 * g + 2) * _H],
                        in0=x0, in1=x1, s0=bpos[:, kc - 1:kc],
                        s1=bpos[:, kb - 1:kb], imm2=4096.0)
                # gather the 4 scan tails into the persistent accumulator
                nc.vector.tensor_copy(pairacc[:, t * 4:(t + 1) * 4], stail)
                for i, k in enumerate((7, 8, 9)):
                    nc.scalar.activation(
                        ascr[:], xt[:], Act.Sign, bias=bpos[:, k - 1:k],
                        scale=-1.0, accum_out=sgnacc[:, t * 3 + i:t * 3 + i + 1])

            # one-tile software pipeline (see module docstring)
            prev = None
            for t in range(T):
                xt = xpool.tile([_P, _D], bf16, tag="xt")
                nc.sync.dma_start(out=xt[:], in_=x[t * _P:(t + 1) * _P, :])

                mmout = mmpool.tile([_P, 2 * _H], f32, tag="mmout")
                delta = small.tile([_P, 1], f32, tag="delta")
                bpos = small.tile([_P, 9], f32, tag="bpos")

                nc.vector._custom_dve(
                    smax, out=mmout[:, 0:_H],
                    in0=xt[:, 0:_H], in1=xt[:, _H:_D])
                nc.vector._custom_dve(
                    smin, out=mmout[:, _H:2 * _H],
                    in0=xt[:, 0:_H], in1=xt[:, _H:_D], s0=fltmax[:])
                mx = mmout[:, _H - 1:_H]
                mn = mmout[:, 2 * _H - 1:2 * _H]
                # boundary math on ACT: delta = -mn + mx ; b_k = frac_k*delta + mn
                nc.scalar.activation(delta[:], mn, Act.Identity,
                                     bias=mx, scale=-1.0)
                nc.scalar.activation(bpos[:], fracs[:], Act.Identity,
                                     bias=mn, scale=delta[:])

                if prev is not None:
                    counts_for(t - 1, prev)
                prev = (xt, bpos)
            counts_for(T - 1, prev)

            # ---- epilogue ----
            # unpack scan tails: hi = round(r * 2^-12) via magic float, exact
            # (r = c_full + 4096*c_half, c_full <= 2047 < 4096/2 rounds down)
            chi = singles.tile([_P, T * 4], f32)
            clo = singles.tile([_P, T * 4], f32)
            nc.vector.tensor_scalar(chi[:], pairacc[:], float(2.0 ** -12),
                                    _MAGIC, Alu.mult, Alu.add)
            nc.vector.tensor_scalar(chi[:], chi[:], -_MAGIC, None, Alu.add)
            nc.vector.scalar_tensor_tensor(
                out=clo[:], in0=chi[:], scalar=-4096.0, in1=pairacc[:],
                op0=Alu.mult, op1=Alu.add)
            chi3 = chi[:].rearrange("p (t s) -> p t s", s=4)
            clo3 = clo[:].rearrange("p (t s) -> p t s", s=4)
            # c1,c3 from ops 0,1; c4,c6 from ops 2,3; c2 = hi0+hi1; c5 = hi2+hi3
            nc.vector.tensor_copy(c3[:, :, 1:2], clo3[:, :, 0:1])
            nc.vector.tensor_copy(c3[:, :, 3:4], clo3[:, :, 1:2])
            nc.vector.tensor_copy(c3[:, :, 4:5], clo3[:, :, 2:3])
            nc.vector.tensor_copy(c3[:, :, 6:7], clo3[:, :, 3:4])
            nc.vector.tensor_tensor(out=c3[:, :, 2:3], in0=chi3[:, :, 0:1],
                                    in1=chi3[:, :, 1:2], op=Alu.add)
            nc.vector.tensor_tensor(out=c3[:, :, 5:6], in0=chi3[:, :, 2:3],
                                    in1=chi3[:, :, 3:4], op=Alu.add)
            # ACT sign-sums (sign(b_k - x)) -> counts: c = 1024 - 0.5*S
            conv = singles.tile([_P, T * 3], f32)
            nc.vector.tensor_scalar(conv[:], sgnacc[:], -0.5, float(_D // 2),
                                    Alu.mult, Alu.add)
            conv3 = conv[:].rearrange("p (t k) -> p t k", k=3)
            nc.vector.tensor_copy(c3[:, :, 7:10], conv3[:, :, 0:3])
            # obs_j = c_j - c_{j+1}
            obs = singles.tile([_P, T * 10], f32)
            obs3 = obs[:].rearrange("p (t j) -> p t j", j=10)
            nc.vector.tensor_tensor(out=obs3[:, :, 0:10], in0=c3[:, :, 0:10],
                                    in1=c3[:, :, 1:11], op=Alu.subtract)

            sq = singles.tile([_P, T * 10], f32)
            part = singles.tile([_P, 1], f32)
            nc.scalar.activation(sq[:], obs[:], Act.Square,
                                 bias=ebias[:], scale=1.0,
                                 accum_out=part[:])
            nc.sync.dma_start(out=out[:], in_=part[:])

    nc.compile()
    return nc


def _get_program():
    if "nc" not in _CACHE:
        _CACHE["nc"] = _build_program()
    return _CACHE["nc"]


def kernel(embeddings: np.ndarray) -> np.ndarray:
    import ml_dtypes
    from concourse.bass_utils import run_bass_kernel_spmd

    assert embeddings.shape == (_B_FULL, _D), embeddings.shape
    x = np.ascontiguousarray(
        embeddings.astype(np.float32).astype(ml_dtypes.bfloat16))
    nc = _get_program()
    in_maps = [
        {"x": x[c * _ROWS_PER_CORE:(c + 1) * _ROWS_PER_CORE]}
        for c in range(_N_CORES)
    ]
    res = run_bass_kernel_spmd(nc, in_maps, core_ids=list(range(_N_CORES)))
    total = np.float64(0.0)
    for r in res.results:
        total += r["partial"].astype(np.float64).sum()
    mean_chi2 = total / np.float64(_E_F32) / np.float64(_B_FULL)
    return np.float32(mean_chi2)


# revision 12
# speedup vs baseline: 2.6236x; 1.0302x over previous
"""ChiSquareLoss kernel for Trainium2 (8 NeuronCores, SPMD).

Problem (see reference): for each of B=16384 rows of a [B, 2048] f32 matrix,
build a 10-bin histogram between the row's min and max, then
chi2_row = sum_j (obs_j - e)^2 / (e + eps) with e = B/10, and return
mean(chi2_row).

Counts per row: c_k = #{x : x > b_k}, b_k = mn + (mx-mn)*k/10 (k=1..9);
obs_j = c_j - c_{j+1}, c_0 = 2048, c_10 = 0.

HW law (measured): any DVE/ACT instruction with an accumulated [P,1]
output runs at 1 output-position per cycle per lane, regardless of
dtype.  The lever is therefore to SHORTEN the accumulated stream and to
PACK counts, via custom DVE microcode ops (concourse.dve_ops extension
point, registered at build time):

  SCANTRIPLE_A : out = scan(+, (S0>C0) + (S1>C0) + (S0>C1)*4096)
  SCANTRIPLE_B : out = scan(+, (S0>C0) + (S1>C0) + (S1>C1)*4096)
  SCANMAX2     : out = scan(max, max(S0, S1))
  SCANMIN2     : out = scan(min, min(S0, S1), init=+FLT_MAX via s0)

Feeding S0/S1 the two halves of a row makes the streamed length 1024
instead of 2048, and the scan's last element IS the fold result -- read
as an AP slice, no accumulator stage or read needed.  One A/B op pair
counts three boundaries (two full + one split across the halves) in
2 x ~1.22us = ~0.82us/count; min/max cost ~1.22us each (vs ~2.4us for
the stock accumulate).  The scan sums stay < 2^23 so fp32 is exact.

Engine split per [128, 2048] bf16 tile (16 tiles/core):
  DVE: SCANMAX2, SCANMIN2, 4 triple ops (boundaries 1..6), and one
       strided [P,4] gather of the scan tails into the pair accumulator.
  ACT: delta = mx-mn, b_k vector, 3 Sign(b_k - x) counts (k=7,8,9).
Both engines ~7.5-7.9us/tile.  One-tile software pipeline: tile t's
counts are emitted after tile t+1's minmax/boundary ops so the
cross-engine chain never stalls either engine.

Input is host-cast to bf16 (classification of bf16(x) against f32
boundaries; measured end-to-end rel err ~1e-5, gate is 2e-2).
Epilogue: unpack triples (2^23 magic-float floor), convert sign-sums,
difference into obs, one ACT Square(obs - e) accumulated pass ->
per-partition partials; host: total / (e + eps) / B.
"""

import numpy as np

_B_FULL = 16384
_D = 2048
_H = _D // 2  # 1024, stream length
_N_CORES = 8
_ROWS_PER_CORE = _B_FULL // _N_CORES  # 2048
_P = 128
_TILES = _ROWS_PER_CORE // _P  # 16
_BINS = 10
_E_F32 = np.float32(_B_FULL / _BINS)  # 1638.4f
_MAGIC = float(np.float32(2 ** 23 + 2 ** 22))  # round-to-int magic for fp32
_FLT_MAX = float(np.finfo(np.float32).max)

_CACHE = {}


def _register_ops():
    """Register the custom DVE scan ops in concourse.dve_ops (documented
    extension point: append to OPS).  sha computed at registration so it
    can never drift.  Idempotent."""
    import concourse.dve_ops as dve_ops
    from concourse.dve_ops import DveOp, OPS, _SUB_OPCODE_FOR_NAME, \
        _CUSTOM_DVE_ROW_BASE
    from concourse.dve_spec import Spec, Src0, Src1, C0, C1, C2, AluOp, \
        maxx, minn, scan, lower
    from concourse.dve_uop import DveOpSpec

    def _reg(name, spec):
        if name in _SUB_OPCODE_FOR_NAME:
            return next(op for op in OPS if op.name == name)
        row = _CUSTOM_DVE_ROW_BASE + len(OPS)
        shas = {
            ver: DveOpSpec(name=name, opcode=row, uops=lower(spec, ver=ver),
                           rd1_en=True).sha(ver)
            for ver in ("v3", "v4")
        }
        op = DveOp(name, spec, subdim=False, uops_sha=shas)
        _SUB_OPCODE_FOR_NAME[name] = row
        OPS.append(op)
        dve_ops.CUSTOM_DVE_SPECS[name] = spec
        return op

    def _r_tri_a(in0, in1, s0, s1, imm2):
        a, b = in0.astype(np.float32), in1.astype(np.float32)
        e = (a > s0) + (b > s0) + (a > s1) * imm2
        return np.cumsum(e.astype(np.float32), -1).astype(np.float32)

    def _r_tri_b(in0, in1, s0, s1, imm2):
        a, b = in0.astype(np.float32), in1.astype(np.float32)
        e = (a > s0) + (b > s0) + (b > s1) * imm2
        return np.cumsum(e.astype(np.float32), -1).astype(np.float32)

    def _r_max(in0, in1, s0, s1, imm2):
        e = np.maximum(in0.astype(np.float32), in1.astype(np.float32))
        return np.maximum.accumulate(e, -1).astype(np.float32)

    def _r_min(in0, in1, s0, s1, imm2):
        e = np.minimum(np.minimum(in0.astype(np.float32),
                                  in1.astype(np.float32)), s0)
        return np.minimum.accumulate(e, -1).astype(np.float32)

    tri_a = _reg("SCANTRIPLE_A_CHI2", Spec(
        body=scan(AluOp.ADD, (Src0 > C0) + (Src1 > C0) + (Src0 > C1) * C2),
        reference=_r_tri_a))
    tri_b = _reg("SCANTRIPLE_B_CHI2", Spec(
        body=scan(AluOp.ADD, (Src0 > C0) + (Src1 > C0) + (Src1 > C1) * C2),
        reference=_r_tri_b))
    smax = _reg("SCANMAX2_CHI2", Spec(
        body=scan(AluOp.MAX, maxx(Src0, Src1)), reference=_r_max))
    smin = _reg("SCANMIN2_CHI2", Spec(
        body=scan(AluOp.MIN, minn(Src0, Src1), init=C0), reference=_r_min))
    return tri_a, tri_b, smax, smin


def _build_program():
    import concourse.bacc as bacc
    import concourse.mybir as mybir
    import concourse.tile as tile

    f32 = mybir.dt.float32
    bf16 = mybir.dt.bfloat16
    Alu = mybir.AluOpType
    Act = mybir.ActivationFunctionType
    tri_a, tri_b, smax, smin = _register_ops()

    nc = bacc.Bacc(None, target_bir_lowering=False)
    x = nc.dram_tensor("x", [_ROWS_PER_CORE, _D], bf16, kind="ExternalInput")
    out = nc.dram_tensor("total", [1, 1], f32, kind="ExternalOutput")

    T = _TILES
    fr = [float(np.float32(k) / np.float32(10.0)) for k in range(1, 10)]

    with tile.TileContext(nc) as tc:
        with tc.tile_pool(name="singles", bufs=1) as singles, \
             tc.tile_pool(name="xp", bufs=4) as xpool, \
             tc.tile_pool(name="mm", bufs=3) as mmpool, \
             tc.tile_pool(name="small", bufs=3) as small, \
             tc.psum_pool(name="ps", bufs=1) as pspool:

            pairacc = singles.tile([_P, T * 4], f32)   # 4 scan tails / tile
            sgnacc = singles.tile([_P, T * 3], f32)    # 3 sign slots / tile
            c_all = singles.tile([_P, T * 11], f32)    # c_0..c_10 per tile
            fracs = singles.tile([_P, 9], f32)
            ebias = singles.tile([_P, 1], f32)         # -e
            fltmax = singles.tile([_P, 1], f32)        # +FLT_MAX (min-scan seed)
            ones = singles.tile([_P, 1], f32)          # partition-sum weights
            scnt = singles.tile([_P, 4 * _H], f32)     # triple-op out scratch
            ascr = singles.tile([_P, _D], bf16)        # ACT sign out scratch
            c3 = c_all[:].rearrange("p (t k) -> p t k", k=11)
            nc.gpsimd.memset(c3[:, :, 0:1], float(_D))  # c_0 = 2048
            nc.gpsimd.memset(c3[:, :, 10:11], 0.0)      # c_10 = 0
            for i, f in enumerate(fr):
                nc.gpsimd.memset(fracs[:, i:i + 1], f)
            nc.gpsimd.memset(ebias[:], -float(_E_F32))
            nc.gpsimd.memset(fltmax[:], _FLT_MAX)
            nc.gpsimd.memset(ones[:], 1.0)
            stail = scnt[:].rearrange("p (s n) -> p s n", n=_H)[:, :, _H - 1]

            def counts_for(t, st):
                xt, bpos = st
                x0, x1 = xt[:, 0:_H], xt[:, _H:_D]
                # boundaries 1..6 via two A/B triple pairs: (1,2,3), (4,5,6)
                for g, (ka, kb, kc) in enumerate([(1, 2, 3), (4, 5, 6)]):
                    nc.vector._custom_dve(
                        tri_a, out=scnt[:, (2 * g) * _H:(2 * g + 1) * _H],
                        in0=x0, in1=x1, s0=bpos[:, ka - 1:ka],
                        s1=bpos[:, kb - 1:kb], imm2=4096.0)
                    nc.vector._custom_dve(
                        tri_b, out=scnt[:, (2 * g + 1) * _H:(2 * g + 2) * _H],
                        in0=x0, in1=x1, s0=bpos[:, kc - 1:kc],
                        s1=bpos[:, kb - 1:kb], imm2=4096.0)
                # gather the 4 scan tails into the persistent accumulator
                # (on ACT: DVE is the busier engine)
                nc.scalar.copy(pairacc[:, t * 4:(t + 1) * 4], stail)
                for i, k in enumerate((7, 8, 9)):
                    nc.scalar.activation(
                        ascr[:], xt[:], Act.Sign, bias=bpos[:, k - 1:k],
                        scale=-1.0, accum_out=sgnacc[:, t * 3 + i:t * 3 + i + 1])

            # one-tile software pipeline (see module docstring)
            prev = None
            for t in range(T):
                xt = xpool.tile([_P, _D], bf16, tag="xt")
                nc.sync.dma_start(out=xt[:], in_=x[t * _P:(t + 1) * _P, :])

                mmout = mmpool.tile([_P, 2 * _H], f32, tag="mmout")
                delta = small.tile([_P, 1], f32, tag="delta")
                bpos = small.tile([_P, 9], f32, tag="bpos")

                nc.vector._custom_dve(
                    smax, out=mmout[:, 0:_H],
                    in0=xt[:, 0:_H], in1=xt[:, _H:_D])
                nc.vector._custom_dve(
                    smin, out=mmout[:, _H:2 * _H],
                    in0=xt[:, 0:_H], in1=xt[:, _H:_D], s0=fltmax[:])
                mx = mmout[:, _H - 1:_H]
                mn = mmout[:, 2 * _H - 1:2 * _H]
                # boundary math on ACT: delta = -mn + mx ; b_k = frac_k*delta + mn
                nc.scalar.activation(delta[:], mn, Act.Identity,
                                     bias=mx, scale=-1.0)
                nc.scalar.activation(bpos[:], fracs[:], Act.Identity,
                                     bias=mn, scale=delta[:])

                if prev is not None:
                    counts_for(t - 1, prev)
                prev = (xt, bpos)
            counts_for(T - 1, prev)

            # ---- epilogue ----
            # unpack scan tails: hi = round(r * 2^-12) via magic float, exact
            # (r = c_full + 4096*c_half, c_full <= 2047 < 4096/2 rounds down)
            chi = singles.tile([_P, T * 4], f32)
            clo = singles.tile([_P, T * 4], f32)
            nc.vector.tensor_scalar(chi[:], pairacc[:], float(2.0 ** -12),
                                    _MAGIC, Alu.mult, Alu.add)
            nc.vector.tensor_scalar(chi[:], chi[:], -_MAGIC, None, Alu.add)
            nc.vector.scalar_tensor_tensor(
                out=clo[:], in0=chi[:], scalar=-4096.0, in1=pairacc[:],
                op0=Alu.mult, op1=Alu.add)
            chi3 = chi[:].rearrange("p (t s) -> p t s", s=4)
            clo3 = clo[:].rearrange("p (t s) -> p t s", s=4)
            # c1,c3 from ops 0,1; c4,c6 from ops 2,3; c2 = hi0+hi1; c5 = hi2+hi3
            nc.vector.tensor_copy(c3[:, :, 1:2], clo3[:, :, 0:1])
            nc.vector.tensor_copy(c3[:, :, 3:4], clo3[:, :, 1:2])
            nc.vector.tensor_copy(c3[:, :, 4:5], clo3[:, :, 2:3])
            nc.vector.tensor_copy(c3[:, :, 6:7], clo3[:, :, 3:4])
            nc.vector.tensor_tensor(out=c3[:, :, 2:3], in0=chi3[:, :, 0:1],
                                    in1=chi3[:, :, 1:2], op=Alu.add)
            nc.vector.tensor_tensor(out=c3[:, :, 5:6], in0=chi3[:, :, 2:3],
                                    in1=chi3[:, :, 3:4], op=Alu.add)
            # ACT sign-sums (sign(b_k - x)) -> counts: c = 1024 - 0.5*S
            conv = singles.tile([_P, T * 3], f32)
            nc.vector.tensor_scalar(conv[:], sgnacc[:], -0.5, float(_D // 2),
                                    Alu.mult, Alu.add)
            conv3 = conv[:].rearrange("p (t k) -> p t k", k=3)
            nc.vector.tensor_copy(c3[:, :, 7:10], conv3[:, :, 0:3])
            # obs_j = c_j - c_{j+1}
            obs = singles.tile([_P, T * 10], f32)
            obs3 = obs[:].rearrange("p (t j) -> p t j", j=10)
            nc.vector.tensor_tensor(out=obs3[:, :, 0:10], in0=c3[:, :, 0:10],
                                    in1=c3[:, :, 1:11], op=Alu.subtract)

            sq = singles.tile([_P, T * 10], f32)
            part = singles.tile([_P, 1], f32)
            nc.scalar.activation(sq[:], obs[:], Act.Square,
                                 bias=ebias[:], scale=1.0,
                                 accum_out=part[:])
            # partition-sum on the (idle) PE so the output DMA is a single
            # descriptor instead of 128 partition-strided 4B ones
            po = pspool.tile([1, 1], f32)
            nc.tensor.matmul(po[:], ones[:], part[:], start=True, stop=True)
            tot = singles.tile([1, 1], f32)
            nc.vector.tensor_copy(tot[:], po[:])
            nc.sync.dma_start(out=out[:], in_=tot[:])

    nc.compile()
    return nc


def _get_program():
    if "nc" not in _CACHE:
        _CACHE["nc"] = _build_program()
    return _CACHE["nc"]


def kernel(embeddings: np.ndarray) -> np.ndarray:
    import ml_dtypes
    from concourse.bass_utils import run_bass_kernel_spmd

    assert embeddings.shape == (_B_FULL, _D), embeddings.shape
    x = np.ascontiguousarray(
        embeddings.astype(np.float32).astype(ml_dtypes.bfloat16))
    nc = _get_program()
    in_maps = [
        {"x": x[c * _ROWS_PER_CORE:(c + 1) * _ROWS_PER_CORE]}
        for c in range(_N_CORES)
    ]
    res = run_bass_kernel_spmd(nc, in_maps, core_ids=list(range(_N_CORES)))
    total = np.float64(0.0)
    for r in res.results:
        total += np.float64(r["total"].reshape(()))
    mean_chi2 = total / np.float64(_E_F32) / np.float64(_B_FULL)
    return np.float32(mean_chi2)


# revision 17
# speedup vs baseline: 2.6412x; 1.0067x over previous
"""ChiSquareLoss kernel for Trainium2 (8 NeuronCores, SPMD).

Problem (see reference): for each of B=16384 rows of a [B, 2048] f32 matrix,
build a 10-bin histogram between the row's min and max, then
chi2_row = sum_j (obs_j - e)^2 / (e + eps) with e = B/10, and return
mean(chi2_row).

Counts per row: c_k = #{x : x > b_k}, b_k = mn + (mx-mn)*k/10 (k=1..9);
obs_j = c_j - c_{j+1}, c_0 = 2048, c_10 = 0.

HW law (measured): any DVE/ACT instruction with an accumulated [P,1]
output runs at 1 output-position per cycle per lane, regardless of
dtype.  The lever is therefore to SHORTEN the accumulated stream and to
PACK counts, via custom DVE microcode ops (concourse.dve_ops extension
point, registered at build time):

  SCANTRIPLE_A : out = scan(+, (S0>C0) + (S1>C0) + (S0>C1)*4096)
  SCANTRIPLE_B : out = scan(+, (S0>C0) + (S1>C0) + (S1>C1)*4096)
  SCANMAX2     : out = scan(max, max(S0, S1))
  SCANMIN2     : out = scan(min, min(S0, S1), init=+FLT_MAX via s0)

Feeding S0/S1 the two halves of a row makes the streamed length 1024
instead of 2048, and the scan's last element IS the fold result -- read
as an AP slice, no accumulator stage or read needed.  One A/B op pair
counts three boundaries (two full + one split across the halves) in
2 x ~1.22us = ~0.82us/count; min/max cost ~1.22us each (vs ~2.4us for
the stock accumulate).  The scan sums stay < 2^23 so fp32 is exact.

Engine split per [128, 2048] bf16 tile (16 tiles/core):
  DVE: SCANMAX2, SCANMIN2, 4 triple ops (boundaries 1..6), and one
       strided [P,4] gather of the scan tails into the pair accumulator.
  ACT: delta = mx-mn, b_k vector, 3 Sign(b_k - x) counts (k=7,8,9).
Both engines ~7.5-7.9us/tile.  One-tile software pipeline: tile t's
counts are emitted after tile t+1's minmax/boundary ops so the
cross-engine chain never stalls either engine.

Input is host-cast to bf16 (classification of bf16(x) against f32
boundaries; measured end-to-end rel err ~1e-5, gate is 2e-2).
Epilogue: unpack triples (2^23 magic-float floor), convert sign-sums,
difference into obs, one ACT Square(obs - e) accumulated pass ->
per-partition partials; host: total / (e + eps) / B.
"""

import numpy as np

_B_FULL = 16384
_D = 2048
_H = _D // 2  # 1024, stream length
_N_CORES = 8
_ROWS_PER_CORE = _B_FULL // _N_CORES  # 2048
_P = 128
_TILES = _ROWS_PER_CORE // _P  # 16
_BINS = 10
_E_F32 = np.float32(_B_FULL / _BINS)  # 1638.4f
_MAGIC = float(np.float32(2 ** 23 + 2 ** 22))  # round-to-int magic for fp32
_FLT_MAX = float(np.finfo(np.float32).max)

_CACHE = {}


def _register_ops():
    """Register the custom DVE scan ops in concourse.dve_ops (documented
    extension point: append to OPS).  sha computed at registration so it
    can never drift.  Idempotent."""
    import concourse.dve_ops as dve_ops
    from concourse.dve_ops import DveOp, OPS, _SUB_OPCODE_FOR_NAME, \
        _CUSTOM_DVE_ROW_BASE
    from concourse.dve_spec import Spec, Src0, Src1, C0, C1, C2, AluOp, \
        maxx, minn, scan, lower
    from concourse.dve_uop import DveOpSpec

    def _reg(name, spec):
        if name in _SUB_OPCODE_FOR_NAME:
            return next(op for op in OPS if op.name == name)
        row = _CUSTOM_DVE_ROW_BASE + len(OPS)
        shas = {
            ver: DveOpSpec(name=name, opcode=row, uops=lower(spec, ver=ver),
                           rd1_en=True).sha(ver)
            for ver in ("v3", "v4")
        }
        op = DveOp(name, spec, subdim=False, uops_sha=shas)
        _SUB_OPCODE_FOR_NAME[name] = row
        OPS.append(op)
        dve_ops.CUSTOM_DVE_SPECS[name] = spec
        return op

    def _r_tri_a(in0, in1, s0, s1, imm2):
        a, b = in0.astype(np.float32), in1.astype(np.float32)
        e = (a > s0) + (b > s0) + (a > s1) * imm2
        return np.cumsum(e.astype(np.float32), -1).astype(np.float32)

    def _r_tri_b(in0, in1, s0, s1, imm2):
        a, b = in0.astype(np.float32), in1.astype(np.float32)
        e = (a > s0) + (b > s0) + (b > s1) * imm2
        return np.cumsum(e.astype(np.float32), -1).astype(np.float32)

    def _r_max(in0, in1, s0, s1, imm2):
        e = np.maximum(in0.astype(np.float32), in1.astype(np.float32))
        return np.maximum.accumulate(e, -1).astype(np.float32)

    def _r_min(in0, in1, s0, s1, imm2):
        e = np.minimum(np.minimum(in0.astype(np.float32),
                                  in1.astype(np.float32)), s0)
        return np.minimum.accumulate(e, -1).astype(np.float32)

    tri_a = _reg("SCANTRIPLE_A_CHI2", Spec(
        body=scan(AluOp.ADD, (Src0 > C0) + (Src1 > C0) + (Src0 > C1) * C2),
        reference=_r_tri_a))
    tri_b = _reg("SCANTRIPLE_B_CHI2", Spec(
        body=scan(AluOp.ADD, (Src0 > C0) + (Src1 > C0) + (Src1 > C1) * C2),
        reference=_r_tri_b))
    smax = _reg("SCANMAX2_CHI2", Spec(
        body=scan(AluOp.MAX, maxx(Src0, Src1)), reference=_r_max))
    smin = _reg("SCANMIN2_CHI2", Spec(
        body=scan(AluOp.MIN, minn(Src0, Src1), init=C0), reference=_r_min))
    return tri_a, tri_b, smax, smin


def _build_program():
    import concourse.bacc as bacc
    import concourse.mybir as mybir
    import concourse.tile as tile

    f32 = mybir.dt.float32
    bf16 = mybir.dt.bfloat16
    Alu = mybir.AluOpType
    Act = mybir.ActivationFunctionType
    tri_a, tri_b, smax, smin = _register_ops()

    nc = bacc.Bacc(None, target_bir_lowering=False)
    x = nc.dram_tensor("x", [_ROWS_PER_CORE, _D], bf16, kind="ExternalInput")
    out = nc.dram_tensor("total", [1, 1], f32, kind="ExternalOutput")

    T = _TILES
    fr = [float(np.float32(k) / np.float32(10.0)) for k in range(1, 10)]

    with tile.TileContext(nc) as tc:
        with tc.tile_pool(name="singles", bufs=1) as singles, \
             tc.tile_pool(name="xp", bufs=4) as xpool, \
             tc.tile_pool(name="mm", bufs=3) as mmpool, \
             tc.tile_pool(name="small", bufs=3) as small, \
             tc.psum_pool(name="ps", bufs=1) as pspool:

            pairacc = singles.tile([_P, T * 4], f32)   # 4 scan tails / tile
            sgnacc = singles.tile([_P, T * 3], f32)    # 3 sign slots / tile
            c_all = singles.tile([_P, T * 11], f32)    # c_0..c_10 per tile
            fracs = singles.tile([_P, 9], f32)
            ebias = singles.tile([_P, 1], f32)         # -e
            fltmax = singles.tile([_P, 1], f32)        # +FLT_MAX (min-scan seed)
            ones = singles.tile([_P, 1], f32)          # partition-sum weights
            scnt = singles.tile([_P, 4 * _H], f32)     # triple-op out scratch
            ascr = singles.tile([_P, _D], bf16)        # ACT sign out scratch
            c3 = c_all[:].rearrange("p (t k) -> p t k", k=11)
            # start tile 0's load before the constant-setup memsets
            xt0 = xpool.tile([_P, _D], bf16, tag="xt")
            nc.sync.dma_start(out=xt0[:], in_=x[0:_P, :])
            nc.gpsimd.memset(c3[:, :, 0:1], float(_D))  # c_0 = 2048
            nc.gpsimd.memset(c3[:, :, 10:11], 0.0)      # c_10 = 0
            for i, f in enumerate(fr):
                nc.gpsimd.memset(fracs[:, i:i + 1], f)
            nc.gpsimd.memset(ebias[:], -float(_E_F32))
            nc.gpsimd.memset(fltmax[:], _FLT_MAX)
            nc.gpsimd.memset(ones[:], 1.0)
            stail = scnt[:].rearrange("p (s n) -> p s n", n=_H)[:, :, _H - 1]

            def counts_for(t, st):
                xt, bpos = st
                x0, x1 = xt[:, 0:_H], xt[:, _H:_D]
                # boundaries 1..6 via two A/B triple pairs: (1,2,3), (4,5,6)
                for g, (ka, kb, kc) in enumerate([(1, 2, 3), (4, 5, 6)]):
                    nc.vector._custom_dve(
                        tri_a, out=scnt[:, (2 * g) * _H:(2 * g + 1) * _H],
                        in0=x0, in1=x1, s0=bpos[:, ka - 1:ka],
                        s1=bpos[:, kb - 1:kb], imm2=4096.0)
                    nc.vector._custom_dve(
                        tri_b, out=scnt[:, (2 * g + 1) * _H:(2 * g + 2) * _H],
                        in0=x0, in1=x1, s0=bpos[:, kc - 1:kc],
                        s1=bpos[:, kb - 1:kb], imm2=4096.0)
                # gather the 4 scan tails into the persistent accumulator
                nc.vector.tensor_copy(pairacc[:, t * 4:(t + 1) * 4], stail)
                for i, k in enumerate((7, 8, 9)):
                    nc.scalar.activation(
                        ascr[:], xt[:], Act.Sign, bias=bpos[:, k - 1:k],
                        scale=-1.0, accum_out=sgnacc[:, t * 3 + i:t * 3 + i + 1])

            # one-tile software pipeline (see module docstring)
            prev = None
            for t in range(T):
                if t == 0:
                    xt = xt0
                else:
                    xt = xpool.tile([_P, _D], bf16, tag="xt")
                    nc.sync.dma_start(out=xt[:], in_=x[t * _P:(t + 1) * _P, :])

                mmout = mmpool.tile([_P, 2 * _H], f32, tag="mmout")
                delta = small.tile([_P, 1], f32, tag="delta")
                bpos = small.tile([_P, 9], f32, tag="bpos")

                nc.vector._custom_dve(
                    smax, out=mmout[:, 0:_H],
                    in0=xt[:, 0:_H], in1=xt[:, _H:_D])
                nc.vector._custom_dve(
                    smin, out=mmout[:, _H:2 * _H],
                    in0=xt[:, 0:_H], in1=xt[:, _H:_D], s0=fltmax[:])
                mx = mmout[:, _H - 1:_H]
                mn = mmout[:, 2 * _H - 1:2 * _H]
                # boundary math on ACT: delta = -mn + mx ; b_k = frac_k*delta + mn
                nc.scalar.activation(delta[:], mn, Act.Identity,
                                     bias=mx, scale=-1.0)
                nc.scalar.activation(bpos[:], fracs[:], Act.Identity,
                                     bias=mn, scale=delta[:])

                if prev is not None:
                    counts_for(t - 1, prev)
                prev = (xt, bpos)
            counts_for(T - 1, prev)

            # ---- epilogue ----
            # unpack scan tails: hi = round(r * 2^-12) via magic float, exact
            # (r = c_full + 4096*c_half, c_full <= 2047 < 4096/2 rounds down)
            chi = singles.tile([_P, T * 4], f32)
            clo = singles.tile([_P, T * 4], f32)
            magic = singles.tile([_P, 1], f32)
            nmagic = singles.tile([_P, 1], f32)
            nc.gpsimd.memset(magic[:], _MAGIC)
            nc.gpsimd.memset(nmagic[:], -_MAGIC)
            # magic-unpack affines on ACT (DVE is the busier engine)
            nc.scalar.activation(chi[:], pairacc[:], Act.Identity,
                                 bias=magic[:], scale=float(2.0 ** -12))
            nc.scalar.activation(chi[:], chi[:], Act.Identity,
                                 bias=nmagic[:], scale=1.0)
            nc.vector.scalar_tensor_tensor(
                out=clo[:], in0=chi[:], scalar=-4096.0, in1=pairacc[:],
                op0=Alu.mult, op1=Alu.add)
            chi3 = chi[:].rearrange("p (t s) -> p t s", s=4)
            clo3 = clo[:].rearrange("p (t s) -> p t s", s=4)
            # c1,c3 from ops 0,1; c4,c6 from ops 2,3; c2 = hi0+hi1; c5 = hi2+hi3
            nc.vector.tensor_copy(c3[:, :, 1:2], clo3[:, :, 0:1])
            nc.vector.tensor_copy(c3[:, :, 3:4], clo3[:, :, 1:2])
            nc.vector.tensor_copy(c3[:, :, 4:5], clo3[:, :, 2:3])
            nc.vector.tensor_copy(c3[:, :, 6:7], clo3[:, :, 3:4])
            nc.vector.tensor_tensor(out=c3[:, :, 2:3], in0=chi3[:, :, 0:1],
                                    in1=chi3[:, :, 1:2], op=Alu.add)
            nc.vector.tensor_tensor(out=c3[:, :, 5:6], in0=chi3[:, :, 2:3],
                                    in1=chi3[:, :, 3:4], op=Alu.add)
            # ACT sign-sums (sign(b_k - x)) -> counts: c = 1024 - 0.5*S
            conv = singles.tile([_P, T * 3], f32)
            halfd = singles.tile([_P, 1], f32)
            nc.gpsimd.memset(halfd[:], float(_D // 2))
            nc.scalar.activation(conv[:], sgnacc[:], Act.Identity,
                                 bias=halfd[:], scale=-0.5)
            conv3 = conv[:].rearrange("p (t k) -> p t k", k=3)
            nc.vector.tensor_copy(c3[:, :, 7:10], conv3[:, :, 0:3])
            # obs_j = c_j - c_{j+1}
            obs = singles.tile([_P, T * 10], f32)
            obs3 = obs[:].rearrange("p (t j) -> p t j", j=10)
            nc.vector.tensor_tensor(out=obs3[:, :, 0:10], in0=c3[:, :, 0:10],
                                    in1=c3[:, :, 1:11], op=Alu.subtract)

            sq = singles.tile([_P, T * 10], f32)
            part = singles.tile([_P, 1], f32)
            nc.scalar.activation(sq[:], obs[:], Act.Square,
                                 bias=ebias[:], scale=1.0,
                                 accum_out=part[:])
            # partition-sum on the (idle) PE so the output DMA is a single
            # descriptor instead of 128 partition-strided 4B ones
            po = pspool.tile([1, 1], f32)
            nc.tensor.matmul(po[:], ones[:], part[:], start=True, stop=True)
            tot = singles.tile([1, 1], f32)
            nc.vector.tensor_copy(tot[:], po[:])
            nc.sync.dma_start(out=out[:], in_=tot[:])

    nc.compile()
    return nc


def _get_program():
    if "nc" not in _CACHE:
        _CACHE["nc"] = _build_program()
    return _CACHE["nc"]


def kernel(embeddings: np.ndarray) -> np.ndarray:
    import ml_dtypes
    from concourse.bass_utils import run_bass_kernel_spmd

    assert embeddings.shape == (_B_FULL, _D), embeddings.shape
    x = np.ascontiguousarray(
        embeddings.astype(np.float32).astype(ml_dtypes.bfloat16))
    nc = _get_program()
    in_maps = [
        {"x": x[c * _ROWS_PER_CORE:(c + 1) * _ROWS_PER_CORE]}
        for c in range(_N_CORES)
    ]
    res = run_bass_kernel_spmd(nc, in_maps, core_ids=list(range(_N_CORES)))
    total = np.float64(0.0)
    for r in res.results:
        total += np.float64(r["total"].reshape(()))
    mean_chi2 = total / np.float64(_E_F32) / np.float64(_B_FULL)
    return np.float32(mean_chi2)


# revision 19
# speedup vs baseline: 2.6498x; 1.0032x over previous
"""ChiSquareLoss kernel for Trainium2 (8 NeuronCores, SPMD).

Problem (see reference): for each of B=16384 rows of a [B, 2048] f32 matrix,
build a 10-bin histogram between the row's min and max, then
chi2_row = sum_j (obs_j - e)^2 / (e + eps) with e = B/10, and return
mean(chi2_row).

Counts per row: c_k = #{x : x > b_k}, b_k = mn + (mx-mn)*k/10 (k=1..9);
obs_j = c_j - c_{j+1}, c_0 = 2048, c_10 = 0.

HW law (measured): any DVE/ACT instruction with an accumulated [P,1]
output runs at 1 output-position per cycle per lane, regardless of
dtype.  The lever is therefore to SHORTEN the accumulated stream and to
PACK counts, via custom DVE microcode ops (concourse.dve_ops extension
point, registered at build time):

  SCANTRIPLE_A : out = scan(+, (S0>C0) + (S1>C0) + (S0>C1)*4096)
  SCANTRIPLE_B : out = scan(+, (S0>C0) + (S1>C0) + (S1>C1)*4096)
  SCANMAX2     : out = scan(max, max(S0, S1))
  SCANMIN2     : out = scan(min, min(S0, S1), init=+FLT_MAX via s0)

Feeding S0/S1 the two halves of a row makes the streamed length 1024
instead of 2048, and the scan's last element IS the fold result -- read
as an AP slice, no accumulator stage or read needed.  One A/B op pair
counts three boundaries (two full + one split across the halves) in
2 x ~1.22us = ~0.82us/count; min/max cost ~1.22us each (vs ~2.4us for
the stock accumulate).  The scan sums stay < 2^23 so fp32 is exact.

Engine split per [128, 2048] bf16 tile (16 tiles/core):
  DVE: SCANMAX2, SCANMIN2, 4 triple ops (boundaries 1..6), and one
       strided [P,4] gather of the scan tails into the pair accumulator.
  ACT: delta = mx-mn, b_k vector, 3 Sign(b_k - x) counts (k=7,8,9).
Both engines ~7.5-7.9us/tile.  One-tile software pipeline: tile t's
counts are emitted after tile t+1's minmax/boundary ops so the
cross-engine chain never stalls either engine.

Input is host-cast to bf16 (classification of bf16(x) against f32
boundaries; measured end-to-end rel err ~1e-5, gate is 2e-2).
Epilogue: unpack triples (2^23 magic-float floor), convert sign-sums,
difference into obs, one ACT Square(obs - e) accumulated pass ->
per-partition partials; host: total / (e + eps) / B.
"""

import numpy as np

_B_FULL = 16384
_D = 2048
_H = _D // 2  # 1024, stream length
_N_CORES = 8
_ROWS_PER_CORE = _B_FULL // _N_CORES  # 2048
_P = 128
_TILES = _ROWS_PER_CORE // _P  # 16
_BINS = 10
_E_F32 = np.float32(_B_FULL / _BINS)  # 1638.4f
_MAGIC = float(np.float32(2 ** 23 + 2 ** 22))  # round-to-int magic for fp32
_FLT_MAX = float(np.finfo(np.float32).max)

_CACHE = {}


def _register_ops():
    """Register the custom DVE scan ops in concourse.dve_ops (documented
    extension point: append to OPS).  sha computed at registration so it
    can never drift.  Idempotent."""
    import concourse.dve_ops as dve_ops
    from concourse.dve_ops import DveOp, OPS, _SUB_OPCODE_FOR_NAME, \
        _CUSTOM_DVE_ROW_BASE
    from concourse.dve_spec import Spec, Src0, Src1, C0, C1, C2, AluOp, \
        maxx, minn, scan, lower
    from concourse.dve_uop import DveOpSpec

    def _reg(name, spec):
        if name in _SUB_OPCODE_FOR_NAME:
            return next(op for op in OPS if op.name == name)
        row = _CUSTOM_DVE_ROW_BASE + len(OPS)
        shas = {
            ver: DveOpSpec(name=name, opcode=row, uops=lower(spec, ver=ver),
                           rd1_en=True).sha(ver)
            for ver in ("v3", "v4")
        }
        op = DveOp(name, spec, subdim=False, uops_sha=shas)
        _SUB_OPCODE_FOR_NAME[name] = row
        OPS.append(op)
        dve_ops.CUSTOM_DVE_SPECS[name] = spec
        return op

    def _r_tri_a(in0, in1, s0, s1, imm2):
        a, b = in0.astype(np.float32), in1.astype(np.float32)
        e = (a > s0) + (b > s0) + (a > s1) * imm2
        return np.cumsum(e.astype(np.float32), -1).astype(np.float32)

    def _r_tri_b(in0, in1, s0, s1, imm2):
        a, b = in0.astype(np.float32), in1.astype(np.float32)
        e = (a > s0) + (b > s0) + (b > s1) * imm2
        return np.cumsum(e.astype(np.float32), -1).astype(np.float32)

    def _r_max(in0, in1, s0, s1, imm2):
        e = np.maximum(in0.astype(np.float32), in1.astype(np.float32))
        return np.maximum.accumulate(e, -1).astype(np.float32)

    def _r_min(in0, in1, s0, s1, imm2):
        e = np.minimum(np.minimum(in0.astype(np.float32),
                                  in1.astype(np.float32)), s0)
        return np.minimum.accumulate(e, -1).astype(np.float32)

    tri_a = _reg("SCANTRIPLE_A_CHI2", Spec(
        body=scan(AluOp.ADD, (Src0 > C0) + (Src1 > C0) + (Src0 > C1) * C2),
        reference=_r_tri_a))
    tri_b = _reg("SCANTRIPLE_B_CHI2", Spec(
        body=scan(AluOp.ADD, (Src0 > C0) + (Src1 > C0) + (Src1 > C1) * C2),
        reference=_r_tri_b))
    smax = _reg("SCANMAX2_CHI2", Spec(
        body=scan(AluOp.MAX, maxx(Src0, Src1)), reference=_r_max))
    smin = _reg("SCANMIN2_CHI2", Spec(
        body=scan(AluOp.MIN, minn(Src0, Src1), init=C0), reference=_r_min))
    return tri_a, tri_b, smax, smin


def _build_program():
    import concourse.bacc as bacc
    import concourse.mybir as mybir
    import concourse.tile as tile

    f32 = mybir.dt.float32
    bf16 = mybir.dt.bfloat16
    Alu = mybir.AluOpType
    Act = mybir.ActivationFunctionType
    tri_a, tri_b, smax, smin = _register_ops()

    nc = bacc.Bacc(None, target_bir_lowering=False)
    x = nc.dram_tensor("x", [_ROWS_PER_CORE, _D], bf16, kind="ExternalInput")
    out = nc.dram_tensor("total", [1, 1], f32, kind="ExternalOutput")

    T = _TILES
    fr = [float(np.float32(k) / np.float32(10.0)) for k in range(1, 10)]

    with tile.TileContext(nc) as tc:
        with tc.tile_pool(name="singles", bufs=1) as singles, \
             tc.tile_pool(name="xp", bufs=4) as xpool, \
             tc.tile_pool(name="mm", bufs=3) as mmpool, \
             tc.tile_pool(name="small", bufs=3) as small, \
             tc.psum_pool(name="ps", bufs=1) as pspool:

            pairacc = singles.tile([_P, T * 4], f32)   # 4 scan tails / tile
            sgnacc = singles.tile([_P, T * 3], f32)    # 3 sign slots / tile
            c_all = singles.tile([_P, T * 11], f32)    # c_0..c_10 per tile
            fracs = singles.tile([_P, 9], f32)
            ebias = singles.tile([_P, 1], f32)         # -e
            fltmax = singles.tile([_P, 1], f32)        # +FLT_MAX (min-scan seed)
            ones = singles.tile([_P, 1], f32)          # partition-sum weights
            scnt = singles.tile([_P, 8 * _H], f32)     # triple-op out scratch
            # (8 blocks: two tiles' worth, gathered with one [P,8] copy)
            ascr = singles.tile([_P, _D], bf16)        # ACT sign out scratch
            c3 = c_all[:].rearrange("p (t k) -> p t k", k=11)
            # start tile 0's load before the constant-setup memsets
            xt0 = xpool.tile([_P, _D], bf16, tag="xt")
            nc.sync.dma_start(out=xt0[:], in_=x[0:_P, :])
            nc.gpsimd.memset(c3[:, :, 0:1], float(_D))  # c_0 = 2048
            nc.gpsimd.memset(c3[:, :, 10:11], 0.0)      # c_10 = 0
            for i, f in enumerate(fr):
                nc.gpsimd.memset(fracs[:, i:i + 1], f)
            nc.gpsimd.memset(ebias[:], -float(_E_F32))
            nc.gpsimd.memset(fltmax[:], _FLT_MAX)
            nc.gpsimd.memset(ones[:], 1.0)
            stail = scnt[:].rearrange("p (s n) -> p s n", n=_H)[:, :, _H - 1]

            def counts_for(t, st):
                xt, bpos = st
                x0, x1 = xt[:, 0:_H], xt[:, _H:_D]
                base = (t % 2) * 4  # even tiles use blocks 0-3, odd 4-7
                # boundaries 1..6 via two A/B triple pairs: (1,2,3), (4,5,6)
                for g, (ka, kb, kc) in enumerate([(1, 2, 3), (4, 5, 6)]):
                    b0 = base + 2 * g
                    nc.vector._custom_dve(
                        tri_a, out=scnt[:, b0 * _H:(b0 + 1) * _H],
                        in0=x0, in1=x1, s0=bpos[:, ka - 1:ka],
                        s1=bpos[:, kb - 1:kb], imm2=4096.0)
                    nc.vector._custom_dve(
                        tri_b, out=scnt[:, (b0 + 1) * _H:(b0 + 2) * _H],
                        in0=x0, in1=x1, s0=bpos[:, kc - 1:kc],
                        s1=bpos[:, kb - 1:kb], imm2=4096.0)
                # gather two tiles' scan tails with one strided copy
                if t % 2 == 1:
                    nc.vector.tensor_copy(
                        pairacc[:, (t - 1) * 4:(t + 1) * 4], stail)
                for i, k in enumerate((7, 8, 9)):
                    nc.scalar.activation(
                        ascr[:], xt[:], Act.Sign, bias=bpos[:, k - 1:k],
                        scale=-1.0, accum_out=sgnacc[:, t * 3 + i:t * 3 + i + 1])

            # one-tile software pipeline (see module docstring)
            prev = None
            for t in range(T):
                if t == 0:
                    xt = xt0
                else:
                    xt = xpool.tile([_P, _D], bf16, tag="xt")
                    nc.sync.dma_start(out=xt[:], in_=x[t * _P:(t + 1) * _P, :])

                mmout = mmpool.tile([_P, 2 * _H], f32, tag="mmout")
                delta = small.tile([_P, 1], f32, tag="delta")
                bpos = small.tile([_P, 9], f32, tag="bpos")

                nc.vector._custom_dve(
                    smax, out=mmout[:, 0:_H],
                    in0=xt[:, 0:_H], in1=xt[:, _H:_D])
                nc.vector._custom_dve(
                    smin, out=mmout[:, _H:2 * _H],
                    in0=xt[:, 0:_H], in1=xt[:, _H:_D], s0=fltmax[:])
                mx = mmout[:, _H - 1:_H]
                mn = mmout[:, 2 * _H - 1:2 * _H]
                # boundary math on ACT: delta = -mn + mx ; b_k = frac_k*delta + mn
                nc.scalar.activation(delta[:], mn, Act.Identity,
                                     bias=mx, scale=-1.0)
                nc.scalar.activation(bpos[:], fracs[:], Act.Identity,
                                     bias=mn, scale=delta[:])

                if prev is not None:
                    counts_for(t - 1, prev)
                prev = (xt, bpos)
            counts_for(T - 1, prev)

            # ---- epilogue ----
            # unpack scan tails: hi = round(r * 2^-12) via magic float, exact
            # (r = c_full + 4096*c_half, c_full <= 2047 < 4096/2 rounds down)
            chi = singles.tile([_P, T * 4], f32)
            clo = singles.tile([_P, T * 4], f32)
            magic = singles.tile([_P, 1], f32)
            nmagic = singles.tile([_P, 1], f32)
            nc.gpsimd.memset(magic[:], _MAGIC)
            nc.gpsimd.memset(nmagic[:], -_MAGIC)
            # magic-unpack affines on ACT (DVE is the busier engine)
            nc.scalar.activation(chi[:], pairacc[:], Act.Identity,
                                 bias=magic[:], scale=float(2.0 ** -12))
            nc.scalar.activation(chi[:], chi[:], Act.Identity,
                                 bias=nmagic[:], scale=1.0)
            nc.vector.scalar_tensor_tensor(
                out=clo[:], in0=chi[:], scalar=-4096.0, in1=pairacc[:],
                op0=Alu.mult, op1=Alu.add)
            chi3 = chi[:].rearrange("p (t s) -> p t s", s=4)
            clo3 = clo[:].rearrange("p (t s) -> p t s", s=4)
            # c1,c3 from ops 0,1; c4,c6 from ops 2,3; c2 = hi0+hi1; c5 = hi2+hi3
            nc.vector.tensor_copy(c3[:, :, 1:2], clo3[:, :, 0:1])
            nc.vector.tensor_copy(c3[:, :, 3:4], clo3[:, :, 1:2])
            nc.vector.tensor_copy(c3[:, :, 4:5], clo3[:, :, 2:3])
            nc.vector.tensor_copy(c3[:, :, 6:7], clo3[:, :, 3:4])
            nc.vector.tensor_tensor(out=c3[:, :, 2:3], in0=chi3[:, :, 0:1],
                                    in1=chi3[:, :, 1:2], op=Alu.add)
            nc.vector.tensor_tensor(out=c3[:, :, 5:6], in0=chi3[:, :, 2:3],
                                    in1=chi3[:, :, 3:4], op=Alu.add)
            # ACT sign-sums (sign(b_k - x)) -> counts: c = 1024 - 0.5*S
            conv = singles.tile([_P, T * 3], f32)
            halfd = singles.tile([_P, 1], f32)
            nc.gpsimd.memset(halfd[:], float(_D // 2))
            nc.scalar.activation(conv[:], sgnacc[:], Act.Identity,
                                 bias=halfd[:], scale=-0.5)
            conv3 = conv[:].rearrange("p (t k) -> p t k", k=3)
            nc.vector.tensor_copy(c3[:, :, 7:10], conv3[:, :, 0:3])
            # obs_j = c_j - c_{j+1}
            obs = singles.tile([_P, T * 10], f32)
            obs3 = obs[:].rearrange("p (t j) -> p t j", j=10)
            nc.vector.tensor_tensor(out=obs3[:, :, 0:10], in0=c3[:, :, 0:10],
                                    in1=c3[:, :, 1:11], op=Alu.subtract)

            sq = singles.tile([_P, T * 10], f32)
            part = singles.tile([_P, 1], f32)
            nc.scalar.activation(sq[:], obs[:], Act.Square,
                                 bias=ebias[:], scale=1.0,
                                 accum_out=part[:])
            # partition-sum on the (idle) PE so the output DMA is a single
            # descriptor instead of 128 partition-strided 4B ones
            po = pspool.tile([1, 1], f32)
            nc.tensor.matmul(po[:], ones[:], part[:], start=True, stop=True)
            tot = singles.tile([1, 1], f32)
            nc.vector.tensor_copy(tot[:], po[:])
            nc.sync.dma_start(out=out[:], in_=tot[:])

    nc.compile()
    return nc


def _get_program():
    if "nc" not in _CACHE:
        _CACHE["nc"] = _build_program()
    return _CACHE["nc"]


def kernel(embeddings: np.ndarray) -> np.ndarray:
    import ml_dtypes
    from concourse.bass_utils import run_bass_kernel_spmd

    assert embeddings.shape == (_B_FULL, _D), embeddings.shape
    x = np.ascontiguousarray(
        embeddings.astype(np.float32).astype(ml_dtypes.bfloat16))
    nc = _get_program()
    in_maps = [
        {"x": x[c * _ROWS_PER_CORE:(c + 1) * _ROWS_PER_CORE]}
        for c in range(_N_CORES)
    ]
    res = run_bass_kernel_spmd(nc, in_maps, core_ids=list(range(_N_CORES)))
    total = np.float64(0.0)
    for r in res.results:
        total += np.float64(r["total"].reshape(()))
    mean_chi2 = total / np.float64(_E_F32) / np.float64(_B_FULL)
    return np.float32(mean_chi2)
